# revision 14
# baseline (speedup 1.0000x reference)
"""ArcFace logits on 8 Trainium2 NeuronCores (Bass/Tile, model-parallel over classes).

Full inputs -> full output:
    input  [512, 512] f32, label [512] int, weight [100000, 512] f32
    -> logits [512, 100000] f32

Sharding: class dim C=100000 split 8 ways (12500/core).

All normalization is done on the host: each core receives a pre-normalized,
pre-transposed fp16 weight shard wt = (w/||w||).T [512, 12500] and a
replicated exT = (64 * x/||x||).T [512, 512] fp16. The device kernel is a
pure fp16 GEMM with f32 PSUM accumulation producing the scaled cosine slab
in [sample, class] orientation, stored as fp16 (halving both directions of
HBM traffic vs f32). The host concatenates the slabs along the class dim,
casts to f32, and overwrites the 512 label entries with exactly computed
margin-adjusted values (f64 on host).
"""

import math
import os
import sys
import types

import numpy as np

N, D, C = 512, 512, 100000
N_CORES = 8
CS = C // N_CORES  # 12500 classes per core
F = 2500           # classes per superchunk (DMA granularity)
SUB = 500          # classes per PSUM tile (<=512 f32 per bank)

SCALE = 64.0
MARGIN = 0.5
THRESH = math.cos(math.pi - MARGIN)
MM = math.sin(math.pi - MARGIN) * MARGIN


def _ensure_paths():
    for p in ("/opt/trn_rl_repo", "/opt/pypackages"):
        if os.path.isdir(p) and p not in sys.path:
            sys.path.append(p)


def _install_ntff_hook_shim():
    """antenv.axon_hooks is not injected in this image; shim it so
    run_bass_kernel_spmd(trace=True) can register the NTFF profile hook."""
    if "antenv.axon_hooks" in sys.modules:
        return
    try:
        import antenv
    except ImportError:
        return
    mod = types.ModuleType("antenv.axon_hooks")
    hook = [None]
    mod.set_axon_ntff_profile_hook = lambda h: hook.__setitem__(0, h)
    mod.get_axon_ntff_profile_hook = lambda: hook[0]
    sys.modules["antenv.axon_hooks"] = mod
    antenv.axon_hooks = mod
    try:
        from trn_agent_boot.trn_boot import _ntff_profile_via_ctypes

        so = "/opt/axon/libaxon_pjrt.so"
        if os.path.exists(so):
            mod.set_axon_ntff_profile_hook(_ntff_profile_via_ctypes(so))
    except Exception:
        pass


_COMPILED = None


def _build():
    global _COMPILED
    if _COMPILED is not None:
        return _COMPILED

    _ensure_paths()
    _install_ntff_hook_shim()

    from contextlib import ExitStack

    import concourse.bacc as bacc
    import concourse.bass as bass
    import concourse.mybir as mybir
    import concourse.tile as tile

    dt = mybir.dt
    AF = mybir.ActivationFunctionType
    f32 = dt.float32
    f16 = dt.float16

    nc = bacc.Bacc("TRN2", target_bir_lowering=False, debug=False,
                   num_devices=N_CORES)

    wt_ap = nc.dram_tensor("wt", [D, CS], f16, kind="ExternalInput").ap()
    ext_ap = nc.dram_tensor("ext", [D, N], f16, kind="ExternalInput").ap()
    out_ap = nc.dram_tensor("out", [N, CS], f16, kind="ExternalOutput").ap()

    # row d = dk*128 + p
    wt3 = wt_ap.rearrange("(k p) c -> p k c", p=128)
    ext3 = ext_ap.rearrange("(k p) n -> p k n", p=128)
    # row n = nb*128 + p
    out3 = out_ap.rearrange("(b p) c -> p b c", p=128)

    with tile.TileContext(nc) as tc, ExitStack() as ctx:
        persist = ctx.enter_context(tc.tile_pool(name="persist", bufs=1))
        # stationary blocks exT[dk, nb], split into two tiles (dk 0-1 and
        # dk 2-3) so the first matmuls only wait on the first half-load.
        # On sync, ahead of the first weight chunk: the scalar queue's
        # ACT_TABLE_LOAD would delay it and gate the first real matmul.
        ext_a = persist.tile([128, 2 * N], f16, tag="exta")
        ext_b = persist.tile([128, 2 * N], f16, tag="extb")
        nc.sync.dma_start(ext_a[:].rearrange("p (k n) -> p k n", k=2),
                          ext3[:, 0:2, :])
        nc.sync.dma_start(ext_b[:].rearrange("p (k n) -> p k n", k=2),
                          ext3[:, 2:4, :])

        wt_pool = ctx.enter_context(tc.tile_pool(name="wt", bufs=4))
        out_pool = ctx.enter_context(tc.tile_pool(name="outp", bufs=2))
        mpsum = ctx.enter_context(
            tc.tile_pool(name="mpsum", bufs=8, space=bass.MemorySpace.PSUM))

        # PE prewarm: dummy matmuls keep the PE HAM activity window busy
        # while the first weight chunk loads, so the real matmul stream
        # reaches the warm 2.4 GHz clock sooner. Writes into a rotating
        # "ps" buffer so no extra PSUM bank is consumed.
        warm = persist.tile([128, 128], f16, tag="warm")
        nc.vector.memset(warm[:], 0.0)
        wps = mpsum.tile([128, SUB], f32, tag="ps", name="ps")
        for _ in range(30):
            nc.tensor.matmul(wps[:, :128], warm[:, :], warm[:, :],
                             start=True, stop=True)

        # taper both ends: small chunks fill the pipeline fast and keep the
        # final stores off the critical path
        chunks = [500, 750, 1250, 1750, 2000, 2000, 2000, 2000, 250]
        assert sum(chunks) == CS
        c0 = 0
        for fs in chunks:
            wtile = wt_pool.tile([128, 4 * fs], f16, tag="wt", name="wt4")
            for dk in range(4):
                nc.sync.dma_start(wtile[:, dk * fs:(dk + 1) * fs],
                                  wt3[:, dk, c0:c0 + fs])
            # [p, nb*fs + c] fp16
            stile = out_pool.tile([128, 4 * fs], f16, tag="st", name="st4")
            subs = [SUB] * (fs // SUB) + ([fs % SUB] if fs % SUB else [])
            s0 = 0
            for sw in subs:
                for nb in range(4):
                    ps = mpsum.tile([128, SUB], f32, tag="ps", name="ps")
                    for dk in range(4):
                        esb = ext_a if dk < 2 else ext_b
                        ofs = (dk % 2) * N + nb * 128
                        nc.tensor.matmul(
                            ps[:, :sw],
                            esb[:, ofs:ofs + 128],
                            wtile[:, dk * fs + s0:dk * fs + s0 + sw],
                            start=(dk == 0), stop=(dk == 3))
                    dst = stile[:, nb * fs + s0:nb * fs + s0 + sw]
                    if nb % 2 == 0:
                        nc.vector.tensor_copy(dst, ps[:, :sw])
                    else:
                        nc.scalar.activation(dst, ps[:, :sw], AF.Copy)
                s0 += sw
            # split big-chunk stores so writes flow instead of bursting at
            # the end of each chunk (keeps the final stores small too)
            st3 = stile[:].rearrange("p (b c) -> p b c", b=4)
            hw = 1000
            if fs > hw:
                for h0 in range(0, fs, hw):
                    w_ = min(hw, fs - h0)
                    nc.scalar.dma_start(
                        out3[:, :, c0 + h0:c0 + h0 + w_],
                        st3[:, :, h0:h0 + w_])
            else:
                nc.scalar.dma_start(out3[:, :, c0:c0 + fs], st3)
            c0 += fs

    nc.compile()
    _COMPILED = nc
    return nc


def kernel(input, label, weight):
    _ensure_paths()
    nc = _build()

    from concourse.bass_utils import run_bass_kernel_spmd

    x = np.asarray(input, dtype=np.float32)
    w = np.asarray(weight, dtype=np.float32)
    lab = np.asarray(label).astype(np.int64)

    ex = x / np.linalg.norm(x, axis=1, keepdims=True)
    ext = np.ascontiguousarray((SCALE * ex).T.astype(np.float16))

    in_maps = []
    for i in range(N_CORES):
        ws = w[i * CS:(i + 1) * CS]
        wn = ws / np.linalg.norm(ws, axis=1, keepdims=True)
        wt = np.ascontiguousarray(wn.T.astype(np.float16))
        in_maps.append({"wt": wt, "ext": ext})

    trace = bool(int(os.environ.get("ARC_TRACE", "0")))
    try:
        res = run_bass_kernel_spmd(nc, in_maps, core_ids=list(range(N_CORES)),
                                   trace=trace)
    except Exception:
        # A previously wedged device (NRT_EXEC_UNIT_UNRECOVERABLE residue)
        # usually recovers on the next load/execute; retry once.
        import time
        time.sleep(2.0)
        res = run_bass_kernel_spmd(nc, in_maps, core_ids=list(range(N_CORES)),
                                   trace=trace)
    kernel._last = res

    slab = np.concatenate([res.results[i]["out"] for i in range(N_CORES)],
                          axis=1)
    logits = slab.astype(np.float32)

    # exact margin-adjusted label entries on host
    wlab = w[lab]
    ewl = wlab / np.linalg.norm(wlab, axis=1, keepdims=True)
    cosl = np.einsum("nd,nd->n", ex.astype(np.float64), ewl.astype(np.float64))
    cos_c = np.clip(cosl, -1.0 + 1e-7, 1.0 - 1e-7)
    phi = np.where(cosl > THRESH,
                   np.cos(np.arccos(cos_c) + MARGIN),
                   cos_c - MM)
    logits[np.arange(N), lab] = (SCALE * phi).astype(np.float32)
    return logits


# revision 15
# speedup vs baseline: 1.0204x; 1.0204x over previous
"""ArcFace logits on 8 Trainium2 NeuronCores (Bass/Tile, model-parallel over classes).

Full inputs -> full output:
    input  [512, 512] f32, label [512] int, weight [100000, 512] f32
    -> logits [512, 100000] f32

Sharding: class dim C=100000 split 8 ways (12500/core).

All normalization is done on the host: each core receives a pre-normalized,
pre-transposed fp16 weight shard wt = (w/||w||).T [512, 12500] and a
replicated exT = (64 * x/||x||).T [512, 512] fp16. The device kernel is a
pure fp16 GEMM with f32 PSUM accumulation producing the scaled cosine slab
in [sample, class] orientation, stored as fp16 (halving both directions of
HBM traffic vs f32). The host concatenates the slabs along the class dim,
casts to f32, and overwrites the 512 label entries with exactly computed
margin-adjusted values (f64 on host).
"""

import math
import os
import sys
import types

import numpy as np

N, D, C = 512, 512, 100000
N_CORES = 8
CS = C // N_CORES  # 12500 classes per core
F = 2500           # classes per superchunk (DMA granularity)
SUB = 500          # classes per PSUM tile (<=512 f32 per bank)

SCALE = 64.0
MARGIN = 0.5
THRESH = math.cos(math.pi - MARGIN)
MM = math.sin(math.pi - MARGIN) * MARGIN


def _ensure_paths():
    for p in ("/opt/trn_rl_repo", "/opt/pypackages"):
        if os.path.isdir(p) and p not in sys.path:
            sys.path.append(p)


def _install_ntff_hook_shim():
    """antenv.axon_hooks is not injected in this image; shim it so
    run_bass_kernel_spmd(trace=True) can register the NTFF profile hook."""
    if "antenv.axon_hooks" in sys.modules:
        return
    try:
        import antenv
    except ImportError:
        return
    mod = types.ModuleType("antenv.axon_hooks")
    hook = [None]
    mod.set_axon_ntff_profile_hook = lambda h: hook.__setitem__(0, h)
    mod.get_axon_ntff_profile_hook = lambda: hook[0]
    sys.modules["antenv.axon_hooks"] = mod
    antenv.axon_hooks = mod
    try:
        from trn_agent_boot.trn_boot import _ntff_profile_via_ctypes

        so = "/opt/axon/libaxon_pjrt.so"
        if os.path.exists(so):
            mod.set_axon_ntff_profile_hook(_ntff_profile_via_ctypes(so))
    except Exception:
        pass


_COMPILED = None


def _build():
    global _COMPILED
    if _COMPILED is not None:
        return _COMPILED

    _ensure_paths()
    _install_ntff_hook_shim()

    from contextlib import ExitStack

    import concourse.bacc as bacc
    import concourse.bass as bass
    import concourse.mybir as mybir
    import concourse.tile as tile

    dt = mybir.dt
    AF = mybir.ActivationFunctionType
    f32 = dt.float32
    f16 = dt.float16

    nc = bacc.Bacc("TRN2", target_bir_lowering=False, debug=False,
                   num_devices=N_CORES)

    wt_ap = nc.dram_tensor("wt", [D, CS], f16, kind="ExternalInput").ap()
    ext_ap = nc.dram_tensor("ext", [D, N], f16, kind="ExternalInput").ap()
    out_ap = nc.dram_tensor("out", [N, CS], f16, kind="ExternalOutput").ap()

    # row d = dk*128 + p
    wt3 = wt_ap.rearrange("(k p) c -> p k c", p=128)
    ext3 = ext_ap.rearrange("(k p) n -> p k n", p=128)
    # row n = nb*128 + p
    out3 = out_ap.rearrange("(b p) c -> p b c", p=128)

    with tile.TileContext(nc) as tc, ExitStack() as ctx:
        persist = ctx.enter_context(tc.tile_pool(name="persist", bufs=1))
        # [p, dk*N + n] fp16: 16 stationary blocks exT[dk, nb].
        # One sync-queue DMA ahead of the first weight chunk: every extra
        # serial DIRECT2D trigger (~0.65us) delays the first real matmul,
        # and the scalar queue's ACT_TABLE_LOAD would delay it even more.
        ext_sb = persist.tile([128, 4 * N], f16, tag="ext")
        nc.sync.dma_start(ext_sb[:].rearrange("p (k n) -> p k n", k=4),
                          ext3[:, :, :])

        wt_pool = ctx.enter_context(tc.tile_pool(name="wt", bufs=4))
        out_pool = ctx.enter_context(tc.tile_pool(name="outp", bufs=2))
        mpsum = ctx.enter_context(
            tc.tile_pool(name="mpsum", bufs=8, space=bass.MemorySpace.PSUM))

        # PE prewarm: dummy matmuls keep the PE HAM activity window busy
        # while the first weight chunk loads, so the real matmul stream
        # reaches the warm 2.4 GHz clock sooner. Writes into a rotating
        # "ps" buffer so no extra PSUM bank is consumed.
        warm = persist.tile([128, 128], f16, tag="warm")
        nc.vector.memset(warm[:], 0.0)
        wps = mpsum.tile([128, SUB], f32, tag="ps", name="ps")
        for _ in range(40):
            nc.tensor.matmul(wps[:, :128], warm[:, :], warm[:, :],
                             start=True, stop=True)

        # taper both ends: small chunks fill the pipeline fast and keep the
        # final stores off the critical path
        chunks = [500, 750, 1250, 1750, 2000, 2000, 2000, 2000, 250]
        assert sum(chunks) == CS
        c0 = 0
        for fs in chunks:
            wtile = wt_pool.tile([128, 4 * fs], f16, tag="wt", name="wt4")
            for dk in range(4):
                nc.sync.dma_start(wtile[:, dk * fs:(dk + 1) * fs],
                                  wt3[:, dk, c0:c0 + fs])
            # [p, nb*fs + c] fp16
            stile = out_pool.tile([128, 4 * fs], f16, tag="st", name="st4")
            subs = [SUB] * (fs // SUB) + ([fs % SUB] if fs % SUB else [])
            s0 = 0
            for sw in subs:
                for nb in range(4):
                    ps = mpsum.tile([128, SUB], f32, tag="ps", name="ps")
                    for dk in range(4):
                        ofs = dk * N + nb * 128
                        nc.tensor.matmul(
                            ps[:, :sw],
                            ext_sb[:, ofs:ofs + 128],
                            wtile[:, dk * fs + s0:dk * fs + s0 + sw],
                            start=(dk == 0), stop=(dk == 3))
                    dst = stile[:, nb * fs + s0:nb * fs + s0 + sw]
                    if nb % 2 == 0:
                        nc.vector.tensor_copy(dst, ps[:, :sw])
                    else:
                        nc.scalar.activation(dst, ps[:, :sw], AF.Copy)
                s0 += sw
            # split big-chunk stores so writes flow instead of bursting at
            # the end of each chunk (keeps the final stores small too)
            st3 = stile[:].rearrange("p (b c) -> p b c", b=4)
            hw = 1000
            if fs > hw:
                for h0 in range(0, fs, hw):
                    w_ = min(hw, fs - h0)
                    nc.scalar.dma_start(
                        out3[:, :, c0 + h0:c0 + h0 + w_],
                        st3[:, :, h0:h0 + w_])
            else:
                nc.scalar.dma_start(out3[:, :, c0:c0 + fs], st3)
            c0 += fs

    nc.compile()
    _COMPILED = nc
    return nc


def kernel(input, label, weight):
    _ensure_paths()
    nc = _build()

    from concourse.bass_utils import run_bass_kernel_spmd

    x = np.asarray(input, dtype=np.float32)
    w = np.asarray(weight, dtype=np.float32)
    lab = np.asarray(label).astype(np.int64)

    ex = x / np.linalg.norm(x, axis=1, keepdims=True)
    ext = np.ascontiguousarray((SCALE * ex).T.astype(np.float16))

    in_maps = []
    for i in range(N_CORES):
        ws = w[i * CS:(i + 1) * CS]
        wn = ws / np.linalg.norm(ws, axis=1, keepdims=True)
        wt = np.ascontiguousarray(wn.T.astype(np.float16))
        in_maps.append({"wt": wt, "ext": ext})

    trace = bool(int(os.environ.get("ARC_TRACE", "0")))
    try:
        res = run_bass_kernel_spmd(nc, in_maps, core_ids=list(range(N_CORES)),
                                   trace=trace)
    except Exception:
        # A previously wedged device (NRT_EXEC_UNIT_UNRECOVERABLE residue)
        # usually recovers on the next load/execute; retry once.
        import time
        time.sleep(2.0)
        res = run_bass_kernel_spmd(nc, in_maps, core_ids=list(range(N_CORES)),
                                   trace=trace)
    kernel._last = res

    slab = np.concatenate([res.results[i]["out"] for i in range(N_CORES)],
                          axis=1)
    logits = slab.astype(np.float32)

    # exact margin-adjusted label entries on host
    wlab = w[lab]
    ewl = wlab / np.linalg.norm(wlab, axis=1, keepdims=True)
    cosl = np.einsum("nd,nd->n", ex.astype(np.float64), ewl.astype(np.float64))
    cos_c = np.clip(cosl, -1.0 + 1e-7, 1.0 - 1e-7)
    phi = np.where(cosl > THRESH,
                   np.cos(np.arccos(cos_c) + MARGIN),
                   cos_c - MM)
    logits[np.arange(N), lab] = (SCALE * phi).astype(np.float32)
    return logits


# revision 16
# speedup vs baseline: 1.0501x; 1.0290x over previous
"""ArcFace logits on 8 Trainium2 NeuronCores (Bass/Tile, model-parallel over classes).

Full inputs -> full output:
    input  [512, 512] f32, label [512] int, weight [100000, 512] f32
    -> logits [512, 100000] f32

Sharding: class dim C=100000 split 8 ways (12500/core).

All normalization is done on the host: each core receives a pre-normalized,
pre-transposed fp16 weight shard wt = (w/||w||).T [512, 12500] and a
replicated exT = (64 * x/||x||).T [512, 512] fp16. The device kernel is a
pure fp16 GEMM with f32 PSUM accumulation producing the scaled cosine slab
in [sample, class] orientation, stored as fp16 (halving both directions of
HBM traffic vs f32). The host concatenates the slabs along the class dim,
casts to f32, and overwrites the 512 label entries with exactly computed
margin-adjusted values (f64 on host).
"""

import math
import os
import sys
import types

import numpy as np

N, D, C = 512, 512, 100000
N_CORES = 8
CS = C // N_CORES  # 12500 classes per core
F = 2500           # classes per superchunk (DMA granularity)
SUB = 500          # classes per PSUM tile (<=512 f32 per bank)

SCALE = 64.0
MARGIN = 0.5
THRESH = math.cos(math.pi - MARGIN)
MM = math.sin(math.pi - MARGIN) * MARGIN


def _ensure_paths():
    for p in ("/opt/trn_rl_repo", "/opt/pypackages"):
        if os.path.isdir(p) and p not in sys.path:
            sys.path.append(p)


def _install_ntff_hook_shim():
    """antenv.axon_hooks is not injected in this image; shim it so
    run_bass_kernel_spmd(trace=True) can register the NTFF profile hook."""
    if "antenv.axon_hooks" in sys.modules:
        return
    try:
        import antenv
    except ImportError:
        return
    mod = types.ModuleType("antenv.axon_hooks")
    hook = [None]
    mod.set_axon_ntff_profile_hook = lambda h: hook.__setitem__(0, h)
    mod.get_axon_ntff_profile_hook = lambda: hook[0]
    sys.modules["antenv.axon_hooks"] = mod
    antenv.axon_hooks = mod
    try:
        from trn_agent_boot.trn_boot import _ntff_profile_via_ctypes

        so = "/opt/axon/libaxon_pjrt.so"
        if os.path.exists(so):
            mod.set_axon_ntff_profile_hook(_ntff_profile_via_ctypes(so))
    except Exception:
        pass


_COMPILED = None


def _build():
    global _COMPILED
    if _COMPILED is not None:
        return _COMPILED

    _ensure_paths()
    _install_ntff_hook_shim()

    from contextlib import ExitStack

    import concourse.bacc as bacc
    import concourse.bass as bass
    import concourse.mybir as mybir
    import concourse.tile as tile

    dt = mybir.dt
    AF = mybir.ActivationFunctionType
    f32 = dt.float32
    f16 = dt.float16

    nc = bacc.Bacc("TRN2", target_bir_lowering=False, debug=False,
                   num_devices=N_CORES)

    wt_ap = nc.dram_tensor("wt", [D, CS], f16, kind="ExternalInput").ap()
    ext_ap = nc.dram_tensor("ext", [D, N], f16, kind="ExternalInput").ap()
    out_ap = nc.dram_tensor("out", [N, CS], f16, kind="ExternalOutput").ap()

    # row d = dk*128 + p
    wt3 = wt_ap.rearrange("(k p) c -> p k c", p=128)
    ext3 = ext_ap.rearrange("(k p) n -> p k n", p=128)
    # row n = nb*128 + p
    out3 = out_ap.rearrange("(b p) c -> p b c", p=128)

    with tile.TileContext(nc) as tc, ExitStack() as ctx:
        persist = ctx.enter_context(tc.tile_pool(name="persist", bufs=1))
        # [p, dk*N + n] fp16: 16 stationary blocks exT[dk, nb].
        # One sync-queue DMA ahead of the first weight chunk: every extra
        # serial DIRECT2D trigger (~0.65us) delays the first real matmul,
        # and the scalar queue's ACT_TABLE_LOAD would delay it even more.
        ext_sb = persist.tile([128, 4 * N], f16, tag="ext")
        nc.sync.dma_start(ext_sb[:].rearrange("p (k n) -> p k n", k=4),
                          ext3[:, :, :])

        wt_pool = ctx.enter_context(tc.tile_pool(name="wt", bufs=4))
        out_pool = ctx.enter_context(tc.tile_pool(name="outp", bufs=2))
        mpsum = ctx.enter_context(
            tc.tile_pool(name="mpsum", bufs=8, space=bass.MemorySpace.PSUM))

        # PE prewarm: dummy matmuls keep the PE HAM activity window busy
        # while the first weight chunk loads, so the real matmul stream
        # reaches the warm 2.4 GHz clock sooner. Writes into a rotating
        # "ps" buffer so no extra PSUM bank is consumed.
        warm = persist.tile([128, 128], f16, tag="warm")
        nc.vector.memset(warm[:], 0.0)
        wps = mpsum.tile([128, SUB], f32, tag="ps", name="ps")
        for _ in range(40):
            nc.tensor.matmul(wps[:, :128], warm[:, :], warm[:, :],
                             start=True, stop=True)

        # taper both ends: small chunks fill the pipeline fast and keep the
        # final stores off the critical path
        chunks = [500, 1000, 2000, 2000, 2000, 2000, 2000, 750, 250]
        assert sum(chunks) == CS
        c0 = 0
        for fs in chunks:
            wtile = wt_pool.tile([128, 4 * fs], f16, tag="wt", name="wt4")
            for dk in range(4):
                nc.sync.dma_start(wtile[:, dk * fs:(dk + 1) * fs],
                                  wt3[:, dk, c0:c0 + fs])
            # [p, nb*fs + c] fp16
            stile = out_pool.tile([128, 4 * fs], f16, tag="st", name="st4")
            subs = [SUB] * (fs // SUB) + ([fs % SUB] if fs % SUB else [])
            s0 = 0
            for sw in subs:
                for nb in range(4):
                    ps = mpsum.tile([128, SUB], f32, tag="ps", name="ps")
                    for dk in range(4):
                        ofs = dk * N + nb * 128
                        nc.tensor.matmul(
                            ps[:, :sw],
                            ext_sb[:, ofs:ofs + 128],
                            wtile[:, dk * fs + s0:dk * fs + s0 + sw],
                            start=(dk == 0), stop=(dk == 3))
                    dst = stile[:, nb * fs + s0:nb * fs + s0 + sw]
                    if nb % 2 == 0:
                        nc.vector.tensor_copy(dst, ps[:, :sw])
                    else:
                        nc.scalar.activation(dst, ps[:, :sw], AF.Copy)
                s0 += sw
            # split big-chunk stores so writes flow instead of bursting at
            # the end of each chunk (keeps the final stores small too)
            st3 = stile[:].rearrange("p (b c) -> p b c", b=4)
            hw = 1000
            if fs > hw:
                for h0 in range(0, fs, hw):
                    w_ = min(hw, fs - h0)
                    nc.scalar.dma_start(
                        out3[:, :, c0 + h0:c0 + h0 + w_],
                        st3[:, :, h0:h0 + w_])
            else:
                nc.scalar.dma_start(out3[:, :, c0:c0 + fs], st3)
            c0 += fs

    nc.compile()
    _COMPILED = nc
    return nc


def kernel(input, label, weight):
    _ensure_paths()
    nc = _build()

    from concourse.bass_utils import run_bass_kernel_spmd

    x = np.asarray(input, dtype=np.float32)
    w = np.asarray(weight, dtype=np.float32)
    lab = np.asarray(label).astype(np.int64)

    ex = x / np.linalg.norm(x, axis=1, keepdims=True)
    ext = np.ascontiguousarray((SCALE * ex).T.astype(np.float16))

    in_maps = []
    for i in range(N_CORES):
        ws = w[i * CS:(i + 1) * CS]
        wn = ws / np.linalg.norm(ws, axis=1, keepdims=True)
        wt = np.ascontiguousarray(wn.T.astype(np.float16))
        in_maps.append({"wt": wt, "ext": ext})

    trace = bool(int(os.environ.get("ARC_TRACE", "0")))
    try:
        res = run_bass_kernel_spmd(nc, in_maps, core_ids=list(range(N_CORES)),
                                   trace=trace)
    except Exception:
        # A previously wedged device (NRT_EXEC_UNIT_UNRECOVERABLE residue)
        # usually recovers on the next load/execute; retry once.
        import time
        time.sleep(2.0)
        res = run_bass_kernel_spmd(nc, in_maps, core_ids=list(range(N_CORES)),
                                   trace=trace)
    kernel._last = res

    slab = np.concatenate([res.results[i]["out"] for i in range(N_CORES)],
                          axis=1)
    logits = slab.astype(np.float32)

    # exact margin-adjusted label entries on host
    wlab = w[lab]
    ewl = wlab / np.linalg.norm(wlab, axis=1, keepdims=True)
    cosl = np.einsum("nd,nd->n", ex.astype(np.float64), ewl.astype(np.float64))
    cos_c = np.clip(cosl, -1.0 + 1e-7, 1.0 - 1e-7)
    phi = np.where(cosl > THRESH,
                   np.cos(np.arccos(cos_c) + MARGIN),
                   cos_c - MM)
    logits[np.arange(N), lab] = (SCALE * phi).astype(np.float32)
    return logits
